# revision 9
# baseline (speedup 1.0000x reference)
"""Bass/Trainium2 kernel for batched per-expert Linear (einsum "bni,nio->bno" + bias).

Strategy:
  - Shard the n (expert) dimension across the 8 NeuronCores: 8 experts/core.
  - Host-side layout choice: pre-transpose x to (n, d_in, batch) and cast
    x/weight to bf16 (PSUM accumulation stays fp32; measured output error
    ~2.4e-3 relative).  This puts the contraction dim (d_in) on SBUF
    partitions for both matmul operands with fully-contiguous DMAs, and
    halves input HBM traffic.
  - Per core: out[b, n, o] = sum_k xT[n, k, b] * w[n, k, o] + bias[n, o]
    as 256 PE matmuls (lhsT = xT chunk [128k, 128b] stationary,
    rhs = w tile [128k, 512o] moving, accumulate 4 k-tiles in PSUM),
    bias added on the PSUM->SBUF copy (DVE), natural-layout output DMA.
"""

import numpy as np
import ml_dtypes

import concourse.bass as bass
import concourse.bacc as bacc
import concourse.mybir as mybir
from concourse import tile
from concourse.bass_utils import run_bass_kernel_spmd

BF16 = ml_dtypes.bfloat16
N_CORES = 8
N, D_IN, D_OUT, BATCH = 64, 512, 512, 1024
NPC = N // N_CORES  # experts per core
P = 128
KT = D_IN // P  # contraction tiles
BT = BATCH // P  # batch tiles


def _build():
    nc = bacc.Bacc(None, target_bir_lowering=False)
    xt = nc.dram_tensor("xt", [NPC, D_IN, BATCH], mybir.dt.bfloat16, kind="ExternalInput")
    w = nc.dram_tensor("w", [NPC, D_IN, D_OUT], mybir.dt.bfloat16, kind="ExternalInput")
    bias = nc.dram_tensor("bias", [NPC, D_OUT], mybir.dt.float32, kind="ExternalInput")
    out = nc.dram_tensor("out", [BATCH, NPC, D_OUT], mybir.dt.bfloat16, kind="ExternalOutput")

    with tile.TileContext(nc) as tc:
        with (
            tc.tile_pool(name="resident", bufs=1) as resp,
            tc.tile_pool(name="outp", bufs=3) as outp,
            tc.tile_pool(name="psum", bufs=6, space="PSUM") as psump,
        ):
            # Alternate big DMAs across the two HWDGE issue queues (SP + ACT):
            # descriptor issue (~1us per 128-descriptor DMA) on a single queue
            # would serialize behind the transfers.
            dma_engines = [nc.sync, nc.scalar]
            dma_i = 0

            def dma(out_ap, in_ap):
                nonlocal dma_i
                dma_engines[dma_i % 2].dma_start(out_ap, in_ap)
                dma_i += 1

            xs, ws = [], []
            for n in range(NPC):
                xtt = resp.tile([P, KT, BATCH], mybir.dt.bfloat16, name=f"x{n}", tag=f"x{n}")
                wt = resp.tile([P, KT, D_OUT], mybir.dt.bfloat16, name=f"w{n}", tag=f"w{n}")
                if n > 0:
                    xs.append(xtt)
                    ws.append(wt)
                    continue
                # first expert's tiles load before everything else so the PE
                # starts as early as possible
                dma(xtt[:], xt[n].rearrange("(kt p) b -> p kt b", p=P))
                dma(wt[:], w[n].rearrange("(kt p) o -> p kt o", p=P))
                xs.append(xtt)
                ws.append(wt)

            bias_sb = resp.tile([P, NPC, D_OUT], mybir.dt.float32, name="bias_sb", tag="bias_sb")
            bias_ap = bias[:]
            bias_bcast = bass.AP(
                tensor=bias_ap.tensor,
                offset=bias_ap.offset,
                ap=[[0, P], bias_ap.ap[0], bias_ap.ap[1]],
            )
            nc.gpsimd.dma_start(out=bias_sb[:], in_=bias_bcast)

            for n in range(1, NPC):
                dma(xs[n][:], xt[n].rearrange("(kt p) b -> p kt b", p=P))
                dma(ws[n][:], w[n].rearrange("(kt p) o -> p kt o", p=P))

            for n in range(NPC):
                # one [P, BT, D_OUT] bf16 tile per expert -> a single 1MB store
                # (DMA issue cost is ~1us per dma_start regardless of size)
                ob = outp.tile([P, BT, D_OUT], mybir.dt.bfloat16, name="ob")
                for bt in range(BT):
                    ps = psump.tile([P, D_OUT], mybir.dt.float32, name="ps")
                    for kt in range(KT):
                        nc.tensor.matmul(
                            ps[:],
                            xs[n][:, kt, bass.ts(bt, P)],
                            ws[n][:, kt, :],
                            start=(kt == 0),
                            stop=(kt == KT - 1),
                        )
                    nc.vector.tensor_add(ob[:, bt, :], ps[:], bias_sb[:, n, :])
                dma(out[:, n, :].rearrange("(bt p) o -> p bt o", p=P), ob[:])
    nc.compile()
    return nc


_NC = None


def _get_nc():
    global _NC
    if _NC is None:
        _NC = _build()
    return _NC


def _run(x, weight, bias, **run_kwargs):
    xt_full = np.ascontiguousarray(np.transpose(x, (1, 2, 0))).astype(BF16)
    w_bf = weight.astype(BF16)
    bias = np.ascontiguousarray(bias, dtype=np.float32)
    in_maps = []
    for c in range(N_CORES):
        sl = slice(c * NPC, (c + 1) * NPC)
        in_maps.append(
            {
                "xt": np.ascontiguousarray(xt_full[sl]),
                "w": np.ascontiguousarray(w_bf[sl]),
                "bias": np.ascontiguousarray(bias[sl]),
            }
        )
    res = run_bass_kernel_spmd(_get_nc(), in_maps, core_ids=list(range(N_CORES)), **run_kwargs)
    out = np.empty((BATCH, N, D_OUT), dtype=np.float32)
    for c in range(N_CORES):
        out[:, c * NPC : (c + 1) * NPC, :] = res.results[c]["out"].astype(np.float32)
    return out, res


def kernel(x, weight, bias):
    out, _ = _run(x, weight, bias)
    return out


# revision 10
# speedup vs baseline: 1.2063x; 1.2063x over previous
"""Bass/Trainium2 kernel for batched per-expert Linear (einsum "bni,nio->bno" + bias).

Strategy:
  - Shard the n (expert) dimension across the 8 NeuronCores: 8 experts/core.
  - Host-side layout choice: pre-transpose x to (n, d_in, batch) and cast
    x/weight to bf16 (PSUM accumulation stays fp32; measured output error
    ~2.4e-3 relative).  This puts the contraction dim (d_in) on SBUF
    partitions for both matmul operands with fully-contiguous DMAs, and
    halves input HBM traffic.
  - Per core: out[b, n, o] = sum_k xT[n, k, b] * w[n, k, o] + bias[n, o]
    as 256 PE matmuls (lhsT = xT chunk [128k, 128b] stationary,
    rhs = w tile [128k, 512o] moving, accumulate 4 k-tiles in PSUM),
    bias added on the PSUM->SBUF copy (DVE), natural-layout output DMA.
"""

import numpy as np
import ml_dtypes

import concourse.bass as bass
import concourse.bacc as bacc
import concourse.mybir as mybir
from concourse import tile
from concourse.bass_utils import run_bass_kernel_spmd

BF16 = ml_dtypes.bfloat16
N_CORES = 8
N, D_IN, D_OUT, BATCH = 64, 512, 512, 1024
NPC = N // N_CORES  # experts per core
P = 128
KT = D_IN // P  # contraction tiles
BT = BATCH // P  # batch tiles


def _build():
    nc = bacc.Bacc(None, target_bir_lowering=False)
    xt = nc.dram_tensor("xt", [NPC, D_IN, BATCH], mybir.dt.bfloat16, kind="ExternalInput")
    w = nc.dram_tensor("w", [NPC, D_IN, D_OUT], mybir.dt.bfloat16, kind="ExternalInput")
    bias = nc.dram_tensor("bias", [NPC, D_OUT], mybir.dt.float32, kind="ExternalInput")
    out = nc.dram_tensor("out", [BATCH, NPC, D_OUT], mybir.dt.bfloat16, kind="ExternalOutput")

    with tile.TileContext(nc) as tc:
        with (
            tc.tile_pool(name="resident", bufs=1) as resp,
            tc.tile_pool(name="outp", bufs=6) as outp,
            tc.tile_pool(name="psum", bufs=6, space="PSUM") as psump,
        ):
            # Two HWDGE issue queues (SP + ACT) share the 16 SDMA engines.
            # Keep each expert's x+w adjacent on ONE queue (alternating per
            # expert) so an expert's weights are never starved behind the
            # other experts' activations on a different queue.
            qs = [nc.sync, nc.scalar]

            xs, ws = [], []
            for n in range(NPC):
                xtt = resp.tile([P, KT, BATCH], mybir.dt.bfloat16, name=f"x{n}", tag=f"x{n}")
                wt = resp.tile([P, KT, D_OUT], mybir.dt.bfloat16, name=f"w{n}", tag=f"w{n}")
                q = qs[n % 2]
                q.dma_start(xtt[:], xt[n].rearrange("(kt p) b -> p kt b", p=P))
                q.dma_start(wt[:], w[n].rearrange("(kt p) o -> p kt o", p=P))
                xs.append(xtt)
                ws.append(wt)

            bias_sb = resp.tile([P, NPC, D_OUT], mybir.dt.float32, name="bias_sb", tag="bias_sb")
            bias_ap = bias[:]
            bias_bcast = bass.AP(
                tensor=bias_ap.tensor,
                offset=bias_ap.offset,
                ap=[[0, P], bias_ap.ap[0], bias_ap.ap[1]],
            )
            nc.gpsimd.dma_start(out=bias_sb[:], in_=bias_bcast)

            HB = BT // 2  # half-batch store granularity
            st_i = 0
            for n in range(NPC):
                for half in range(2):
                    ob = outp.tile([P, HB, D_OUT], mybir.dt.bfloat16, name="ob")
                    for hb in range(HB):
                        bt = half * HB + hb
                        ps = psump.tile([P, D_OUT], mybir.dt.float32, name="ps")
                        for kt in range(KT):
                            nc.tensor.matmul(
                                ps[:],
                                xs[n][:, kt, bass.ts(bt, P)],
                                ws[n][:, kt, :],
                                start=(kt == 0),
                                stop=(kt == KT - 1),
                            )
                        nc.vector.tensor_add(ob[:, hb, :], ps[:], bias_sb[:, n, :])
                    dst = out[bass.ds(half * HB * P, HB * P), n, :]
                    qs[st_i % 2].dma_start(dst.rearrange("(bt p) o -> p bt o", p=P), ob[:])
                    st_i += 1
    nc.compile()
    return nc


_NC = None


def _get_nc():
    global _NC
    if _NC is None:
        _NC = _build()
    return _NC


def _run(x, weight, bias, **run_kwargs):
    xt_full = np.ascontiguousarray(np.transpose(x, (1, 2, 0))).astype(BF16)
    w_bf = weight.astype(BF16)
    bias = np.ascontiguousarray(bias, dtype=np.float32)
    in_maps = []
    for c in range(N_CORES):
        sl = slice(c * NPC, (c + 1) * NPC)
        in_maps.append(
            {
                "xt": np.ascontiguousarray(xt_full[sl]),
                "w": np.ascontiguousarray(w_bf[sl]),
                "bias": np.ascontiguousarray(bias[sl]),
            }
        )
    res = run_bass_kernel_spmd(_get_nc(), in_maps, core_ids=list(range(N_CORES)), **run_kwargs)
    out = np.empty((BATCH, N, D_OUT), dtype=np.float32)
    for c in range(N_CORES):
        out[:, c * NPC : (c + 1) * NPC, :] = res.results[c]["out"].astype(np.float32)
    return out, res


def kernel(x, weight, bias):
    out, _ = _run(x, weight, bias)
    return out
